# revision 6
# baseline (speedup 1.0000x reference)
"""LinearAttention Trainium2 kernel (8 NeuronCores, sequence-sharded).

Reference computation (per batch b):
    qkv = x @ W_qkv; q,k,v split; per-head: softmax(q, dim=dh),
    softmax(k, dim=seq); ctx = k^T v; out = q_sm @ ctx; y = out @ W_out + b.

Device dataflow per core (sequence shard of 1024 rows x 2 batches):
  phase K (per batch): kv = xT.T @ Wkv per 128-row tile; exp_k; per-head
           ctxT_h = v_h^T exp_k_h and Z[d] = colsum(exp_k) accumulated in
           SBUF; flushed as bf16 and AllGathered across the 8 cores.
           (k,v only -- q is deferred so the collective flies early.)
  phase Q (while collectives run): qT = Wq^T @ xT directly in transposed
           layout (no PE transposes); exp; per-head sums via ones-matmuls
           (contract over partitions); reciprocal broadcast back via a
           selector matmul; qsmT = exp_qT * (1/qsum) in bf16.
  phase 45 (per batch): sum the 8 gathered [ctxT|Z] shards locally,
           fold 1/Z into qsmT; M_h = ctx_h @ W_out_h; y = sum_t
           qsmZT_t.T @ M_t; y stored bf16.
Host: tiles/transposes/casts x, splits W_qkv, upcasts y + adds b_out.
"""
import numpy as np
import ml_dtypes
from contextlib import ExitStack

import concourse.bass as bass
import concourse.mybir as mybir
import concourse.tile as tile
from concourse import bacc
from concourse.bass_utils import run_bass_kernel_spmd

bf16 = ml_dtypes.bfloat16
F32 = mybir.dt.float32
BF = mybir.dt.bfloat16
EXP = mybir.ActivationFunctionType.Exp

B, N, D = 2, 8192, 1024
H, DH, INNER = 8, 64, 512
NCORES = 8
NL = N // NCORES            # 1024 seq rows per batch per core
SEQ = B * NL                # 2048 rows per core
NT_B = NL // 128            # 8 seq tiles per batch
NT = B * NT_B               # 16 seq tiles per core
NCH = SEQ // 512            # 4 seq chunks of 512 for the q phase


def _body(tc, xt, wq, wkv, wo, osel_d, bsel_d, y):
    nc = tc.nc
    with ExitStack() as ctx:
        const = ctx.enter_context(tc.tile_pool(name="const", bufs=1))
        dram = ctx.enter_context(tc.tile_pool(name="dram", bufs=1, space="DRAM"))

        ones_bf = const.tile([128, 1], BF)
        nc.vector.memset(ones_bf, 1.0)
        # osel[:, t, j]: contract exp_qT block t -> per-head col sums at
        # psum partition j (j = 2t + (p>=64)).  Host-provided.
        osel = const.tile([128, 4, 8], BF)
        nc.sync.dma_start(out=osel, in_=osel_d[:])
        # bsel[j, t, m]: broadcast per-head scalar j back to the 128
        # partitions of block t (head 2t -> rows 0:64, 2t+1 -> 64:128).
        bsel = const.tile([8, 4, 128], BF)
        nc.sync.dma_start(out=bsel, in_=bsel_d[:])

        wkv_sb = const.tile([128, 8, 2 * INNER], BF)
        for kk in range(8):
            nc.sync.dma_start(out=wkv_sb[:, kk, :], in_=wkv[128 * kk:128 * (kk + 1), :])
        wq_sb = const.tile([128, 8, INNER], BF)
        for kk in range(8):
            nc.sync.dma_start(out=wq_sb[:, kk, :], in_=wq[128 * kk:128 * (kk + 1), :])
        wo_sb = const.tile([128, 4, D], BF)
        for t in range(4):
            nc.sync.dma_start(out=wo_sb[:, t, :], in_=wo[128 * t:128 * (t + 1), :])

        # full x shard resident: [d-part, tile, ktile, seq]
        xfull = const.tile([128, NT, 8, 128], BF)
        for m in range(NT):
            nc.sync.dma_start(out=xfull[:, m, :, :],
                              in_=xt[:, m * 1024:(m + 1) * 1024])

        qsmT = const.tile([128, 4, SEQ], BF)  # persistent q_sm^T (no 1/Z yet)

        red = []  # AllGathered [ctxT | Z] bf16 per batch: [8*128, 260]
        with ExitStack() as pk:
            work = pk.enter_context(tc.tile_pool(name="work", bufs=3))
            acc_pool = pk.enter_context(tc.tile_pool(name="acc", bufs=2))
            small = pk.enter_context(tc.tile_pool(name="small", bufs=2))
            kv_psum = pk.enter_context(tc.tile_pool(name="kv_ps", bufs=2, space="PSUM"))
            ctx_psum = pk.enter_context(tc.tile_pool(name="ctx_ps", bufs=2, space="PSUM"))

            for b in range(B):
                cz_acc = acc_pool.tile([128, 260], F32, tag="cz_acc")
                nc.vector.memset(cz_acc, 0.0)
                for mb in range(NT_B):
                    m = b * NT_B + mb
                    kv_ps = kv_psum.tile([128, 1024], F32, tag="kv")
                    for kk in range(8):
                        for half in range(2):
                            nc.tensor.matmul(
                                kv_ps[:, half * 512:(half + 1) * 512],
                                lhsT=xfull[:, m, kk, :],
                                rhs=wkv_sb[:, kk, half * 512:(half + 1) * 512],
                                start=(kk == 0), stop=(kk == 7))

                    expk = work.tile([128, INNER], BF, tag="expk")
                    nc.scalar.activation(out=expk, in_=kv_ps[:, 0:512], func=EXP)
                    vsb = work.tile([128, INNER], BF, tag="v")
                    nc.vector.tensor_copy(out=vsb, in_=kv_ps[:, 512:1024])

                    # per-tile single-shot ctx/Z matmuls, accumulate in SBUF
                    cz = ctx_psum.tile([128, 260], F32, tag="cz")
                    for h in range(H):
                        t, r = h // 2, h % 2
                        nc.tensor.matmul(
                            cz[64 * r:64 * (r + 1), 64 * t:64 * (t + 1)],
                            lhsT=vsb[:, 64 * h:64 * (h + 1)],
                            rhs=expk[:, 64 * h:64 * (h + 1)],
                            start=True, stop=True)
                    # Z4: col sums of exp_k; dims 128j..128j+127 -> col 256+j
                    for j in range(4):
                        nc.tensor.matmul(
                            cz[:, 256 + j:257 + j],
                            lhsT=expk[:, 128 * j:128 * (j + 1)], rhs=ones_bf,
                            start=True, stop=True)
                    nc.vector.tensor_add(cz_acc, cz_acc, cz)

                czb = small.tile([128, 260], BF, tag="czb")
                nc.vector.tensor_copy(out=czb, in_=cz_acc)
                part_b = dram.tile([128, 260], BF, tag=f"part{b}")
                red_b = dram.tile([NCORES * 128, 260], BF, tag=f"red{b}",
                                  addr_space="Shared")
                nc.sync.dma_start(out=part_b, in_=czb)
                nc.gpsimd.collective_compute(
                    "AllGather", mybir.AluOpType.bypass,
                    replica_groups=[list(range(NCORES))],
                    ins=[part_b.opt()], outs=[red_b.opt()])
                red.append(red_b)

        # phase Q: q^T GEMM + softmax in transposed layout (local only;
        # overlaps the AllGathers). Chunks 0,1 cover batch 0.
        with ExitStack() as pq:
            eq_pool = pq.enter_context(tc.tile_pool(name="eq", bufs=5))
            rq_pool = pq.enter_context(tc.tile_pool(name="rq", bufs=2))
            qt_psum = pq.enter_context(tc.tile_pool(name="qt_ps", bufs=2, space="PSUM"))
            qs_psum = pq.enter_context(tc.tile_pool(name="qs_ps", bufs=2, space="PSUM"))
            bc_psum = pq.enter_context(tc.tile_pool(name="bc_ps", bufs=2, space="PSUM"))

            for c in range(NCH):
                eqs = []
                qsum_ps = qs_psum.tile([8, 512], F32, tag="qsum")
                for t in range(4):
                    qt_ps = qt_psum.tile([128, 512], F32, tag="qt")
                    for kk in range(8):
                        nc.tensor.matmul(
                            qt_ps,
                            lhsT=wq_sb[:, kk, 128 * t:128 * (t + 1)],
                            rhs=xfull[:, 4 * c:4 * (c + 1), kk, :],
                            start=(kk == 0), stop=(kk == 7))
                    eqT = eq_pool.tile([128, 512], BF, tag="eq")
                    nc.scalar.activation(out=eqT, in_=qt_ps, func=EXP)
                    eqs.append(eqT)
                    nc.tensor.matmul(
                        qsum_ps, lhsT=osel[:, t, :], rhs=eqT,
                        start=(t == 0), stop=(t == 3))
                rq = rq_pool.tile([8, 512], BF, tag="rq")
                with nc.allow_low_precision(reason="1/qsum broadcast in bf16"):
                    nc.vector.reciprocal(rq, qsum_ps)
                for t in range(4):
                    bc_ps = bc_psum.tile([128, 512], F32, tag="bc")
                    nc.tensor.matmul(bc_ps, lhsT=bsel[:, t, :], rhs=rq,
                                     start=True, stop=True)
                    nc.vector.tensor_mul(
                        qsmT[:, t, 512 * c:512 * (c + 1)], eqs[t], bc_ps)

        with ExitStack() as p45:
            work2 = p45.enter_context(tc.tile_pool(name="work2", bufs=2))
            small2 = p45.enter_context(tc.tile_pool(name="small2", bufs=2))
            ysb_pool = p45.enter_context(tc.tile_pool(name="ysb", bufs=4))
            m_psum = p45.enter_context(tc.tile_pool(name="m_ps", bufs=2, space="PSUM"))
            y_psum = p45.enter_context(tc.tile_pool(name="y_ps", bufs=4, space="PSUM"))

            for b in range(B):
                allcz = work2.tile([128, 8, 260], BF, tag="allcz")
                nc.sync.dma_start(
                    out=allcz,
                    in_=red[b][:].rearrange("(r p) c -> p r c", p=128))
                # tree-sum the 8 shards in f32
                s0 = work2.tile([128, 260], F32, tag="s0")
                s1 = work2.tile([128, 260], F32, tag="s1")
                s2 = work2.tile([128, 260], F32, tag="s2")
                s3 = work2.tile([128, 260], F32, tag="s3")
                nc.vector.tensor_add(s0, allcz[:, 0, :], allcz[:, 1, :])
                nc.vector.tensor_add(s1, allcz[:, 2, :], allcz[:, 3, :])
                nc.vector.tensor_add(s2, allcz[:, 4, :], allcz[:, 5, :])
                nc.vector.tensor_add(s3, allcz[:, 6, :], allcz[:, 7, :])
                nc.vector.tensor_add(s0, s0, s1)
                nc.vector.tensor_add(s2, s2, s3)
                red_sb = work2.tile([128, 260], F32, tag="red")
                nc.vector.tensor_add(red_sb, s0, s2)

                ctxbf = work2.tile([128, 256], BF, tag="ctxbf")
                nc.vector.tensor_copy(out=ctxbf, in_=red_sb[:, 0:256])
                rz = small2.tile([128, 4], F32, tag="rz")
                nc.vector.reciprocal(rz, red_sb[:, 256:260])

                # fold 1/Z into q_sm^T (per-partition scalar per 128-dim region)
                qsmZ = work2.tile([128, 4, NL], BF, tag="qsmz")
                for t in range(4):
                    nc.vector.tensor_scalar_mul(
                        qsmZ[:, t, :], qsmT[:, t, b * NL:(b + 1) * NL], rz[:, t:t + 1])

                # M_h = ctx_h @ Wout_h ; M region t rows = dims 128t..128t+127
                m_sb = work2.tile([128, 4, D], BF, tag="msb")
                for t in range(4):
                    for cb in range(2):
                        mp = m_psum.tile([128, 512], F32, tag="mp")
                        for r in range(2):
                            nc.tensor.matmul(
                                mp[64 * r:64 * (r + 1), :],
                                lhsT=ctxbf[64 * r:64 * (r + 1), 64 * t:64 * (t + 1)],
                                rhs=wo_sb[64 * r:64 * (r + 1), t, cb * 512:(cb + 1) * 512],
                                start=True, stop=True)
                        nc.vector.tensor_copy(
                            out=m_sb[:, t, cb * 512:(cb + 1) * 512], in_=mp)

                # y = sum_t qsmZ_t^T @ M_t
                for mi in range(NT_B):
                    for cb in range(2):
                        yp = y_psum.tile([128, 512], F32, tag="yp")
                        for t in range(4):
                            nc.tensor.matmul(
                                yp, lhsT=qsmZ[:, t, mi * 128:(mi + 1) * 128],
                                rhs=m_sb[:, t, cb * 512:(cb + 1) * 512],
                                start=(t == 0), stop=(t == 3))
                        ysb = ysb_pool.tile([128, 512], BF, tag="ysb")
                        nc.scalar.copy(out=ysb, in_=yp)
                        nc.sync.dma_start(
                            out=y[b * NL + mi * 128: b * NL + (mi + 1) * 128,
                                  cb * 512:(cb + 1) * 512],
                            in_=ysb)


_COMPILED = None


def _build():
    global _COMPILED
    if _COMPILED is None:
        nc = bacc.Bacc("TRN2", target_bir_lowering=False, debug=False,
                       num_devices=NCORES)
        xt = nc.declare_dram_parameter("xt", [128, NT * 1024], BF, isOutput=False)
        wq = nc.declare_dram_parameter("wq", [D, INNER], BF, isOutput=False)
        wkv = nc.declare_dram_parameter("wkv", [D, 2 * INNER], BF, isOutput=False)
        wo = nc.declare_dram_parameter("wo", [INNER, D], BF, isOutput=False)
        osel_d = nc.declare_dram_parameter("osel", [128, 32], BF, isOutput=False)
        bsel_d = nc.declare_dram_parameter("bsel", [8, 512], BF, isOutput=False)
        y = nc.declare_dram_parameter("y", [SEQ, D], BF, isOutput=True)
        with tile.TileContext(nc) as tc:
            _body(tc, xt, wq, wkv, wo, osel_d, bsel_d, y)
        nc.compile()
        _COMPILED = nc
    return _COMPILED


def _sel_arrays():
    osel = np.zeros((128, 4, 8), np.float32)
    bsel = np.zeros((8, 4, 128), np.float32)
    for t in range(4):
        osel[0:64, t, 2 * t] = 1.0
        osel[64:128, t, 2 * t + 1] = 1.0
        bsel[2 * t, t, 0:64] = 1.0
        bsel[2 * t + 1, t, 64:128] = 1.0
    return (osel.reshape(128, 32).astype(bf16),
            bsel.reshape(8, 512).astype(bf16))


def _make_in_maps(x, W_qkv, W_out):
    osel_np, bsel_np = _sel_arrays()
    wq_bf = np.ascontiguousarray(W_qkv[:, 0:INNER]).astype(bf16)
    wkv_bf = np.ascontiguousarray(W_qkv[:, INNER:]).astype(bf16)
    wo_bf = np.ascontiguousarray(W_out).astype(bf16)
    in_maps = []
    for c in range(NCORES):
        rows = slice(c * NL, (c + 1) * NL)
        xs = np.concatenate([x[0, rows], x[1, rows]], axis=0)  # [2048, 1024]
        # [seq, d] -> [d-part 128, tile 16, ktile 8, s 128] flattened
        xtl = xs.reshape(NT, 128, 8, 128).transpose(3, 0, 2, 1)
        xtl = np.ascontiguousarray(xtl).astype(bf16).reshape(128, NT * 1024)
        in_maps.append({"xt": xtl, "wq": wq_bf, "wkv": wkv_bf, "wo": wo_bf,
                        "osel": osel_np, "bsel": bsel_np})
    return in_maps


def _run(x, W_qkv, W_out, b_out, trace=False, **spmd_kwargs):
    nc = _build()
    in_maps = _make_in_maps(x, W_qkv, W_out)
    res = run_bass_kernel_spmd(nc, in_maps, list(range(NCORES)),
                               trace=trace, **spmd_kwargs)
    out = np.empty((B, N, D), np.float32)
    for c in range(NCORES):
        yc = np.asarray(res.results[c]["y"]).astype(np.float32)
        rows = slice(c * NL, (c + 1) * NL)
        out[0, rows] = yc[:NL]
        out[1, rows] = yc[NL:]
    out += np.asarray(b_out, np.float32)[None, None, :]
    return out, res


def kernel(x, W_qkv, W_out, b_out):
    x = np.asarray(x, np.float32)
    out, _ = _run(x, np.asarray(W_qkv, np.float32),
                  np.asarray(W_out, np.float32),
                  np.asarray(b_out, np.float32))
    return out


# revision 11
# speedup vs baseline: 1.0788x; 1.0788x over previous
"""LinearAttention Trainium2 kernel (8 NeuronCores, sequence-sharded).

Reference computation (per batch b):
    qkv = x @ W_qkv; q,k,v split; per-head: softmax(q, dim=dh),
    softmax(k, dim=seq); ctx = k^T v; out = q_sm @ ctx; y = out @ W_out + b.

Device dataflow per core (sequence shard of 1024 rows x 2 batches):
  phase K (per batch): kv = xT.T @ Wkv per 128-row tile; exp_k; per-head
           ctxT_h = v_h^T exp_k_h and Z[d] = colsum(exp_k) accumulated in
           SBUF; flushed as bf16 and AllGathered across the 8 cores.
           (k,v only -- q is deferred so the collective flies early.)
  phase Q (while collectives run): qT = Wq^T @ xT directly in transposed
           layout (no PE transposes); exp; per-head sums via ones-matmuls
           (contract over partitions); reciprocal broadcast back via a
           selector matmul; qsmT = exp_qT * (1/qsum) in bf16.
  phase 45 (per batch): sum the 8 gathered [ctxT|Z] shards locally,
           fold 1/Z into qsmT; M_h = ctx_h @ W_out_h; y = sum_t
           qsmZT_t.T @ M_t; y stored bf16.
Host: tiles/transposes/casts x, splits W_qkv, upcasts y + adds b_out.
"""
import numpy as np
import ml_dtypes
from contextlib import ExitStack

import concourse.bass as bass
import concourse.mybir as mybir
import concourse.tile as tile
from concourse import bacc
from concourse.bass_utils import run_bass_kernel_spmd

bf16 = ml_dtypes.bfloat16
F32 = mybir.dt.float32
BF = mybir.dt.bfloat16
EXP = mybir.ActivationFunctionType.Exp

B, N, D = 2, 8192, 1024
H, DH, INNER = 8, 64, 512
NCORES = 8
NL = N // NCORES            # 1024 seq rows per batch per core
SEQ = B * NL                # 2048 rows per core
NT_B = NL // 128            # 8 seq tiles per batch
NT = B * NT_B               # 16 seq tiles per core
NCH = SEQ // 512            # 4 seq chunks of 512 for the q phase


def _body(tc, xt, wq, wkv, wo, osel_d, bsel_d, y):
    nc = tc.nc
    with ExitStack() as ctx:
        const = ctx.enter_context(tc.tile_pool(name="const", bufs=1))
        dram = ctx.enter_context(tc.tile_pool(name="dram", bufs=1, space="DRAM"))

        ones_bf = const.tile([128, 1], BF)
        nc.vector.memset(ones_bf, 1.0)
        # osel[:, t, j]: contract exp_qT block t -> per-head col sums at
        # psum partition j (j = 2t + (p>=64)).  Host-provided.
        osel = const.tile([128, 4, 8], BF)
        nc.sync.dma_start(out=osel, in_=osel_d[:])
        # bsel[j, t, m]: broadcast per-head scalar j back to the 128
        # partitions of block t (head 2t -> rows 0:64, 2t+1 -> 64:128).
        bsel = const.tile([8, 4, 128], BF)
        nc.sync.dma_start(out=bsel, in_=bsel_d[:])

        # DMA issue order matters: the Sync HWDGE queue is FIFO, so stage
        # exactly what tile 0's matmuls need first, then trickle the rest.
        wkv_sb = const.tile([128, 8, 2 * INNER], BF)
        wq_sb = const.tile([128, 8, INNER], BF)
        wo_sb = const.tile([128, 4, D], BF)
        xfull = const.tile([128, NT, 8, 128], BF)  # [d-part, tile, ktile, seq]

        def dma_wkv(kk):
            nc.sync.dma_start(out=wkv_sb[:, kk, :], in_=wkv[128 * kk:128 * (kk + 1), :])

        def dma_x(m):
            nc.sync.dma_start(out=xfull[:, m, :, :],
                              in_=xt[:, m * 1024:(m + 1) * 1024])

        dma_wkv(0)
        dma_x(0)
        for kk in range(1, 8):
            dma_wkv(kk)
        for m in range(1, 4):
            dma_x(m)
        for kk in range(8):
            nc.sync.dma_start(out=wq_sb[:, kk, :], in_=wq[128 * kk:128 * (kk + 1), :])
        for m in range(4, 10):
            dma_x(m)
        for t in range(4):
            nc.sync.dma_start(out=wo_sb[:, t, :], in_=wo[128 * t:128 * (t + 1), :])
        for m in range(10, NT):
            dma_x(m)

        qsmT = const.tile([128, 4, SEQ], BF)  # persistent q_sm^T (no 1/Z yet)

        red = []  # AllGathered [ctxT | Z] bf16 per batch: [8*128, 260]
        with ExitStack() as pk:
            work = pk.enter_context(tc.tile_pool(name="work", bufs=3))
            acc_pool = pk.enter_context(tc.tile_pool(name="acc", bufs=2))
            small = pk.enter_context(tc.tile_pool(name="small", bufs=2))
            kv_psum = pk.enter_context(tc.tile_pool(name="kv_ps", bufs=2, space="PSUM"))
            ctx_psum = pk.enter_context(tc.tile_pool(name="ctx_ps", bufs=2, space="PSUM"))

            for b in range(B):
                cz_acc = acc_pool.tile([128, 260], F32, tag="cz_acc")
                nc.vector.memset(cz_acc, 0.0)
                for mb in range(NT_B):
                    m = b * NT_B + mb
                    kv_ps = kv_psum.tile([128, 1024], F32, tag="kv")
                    for kk in range(8):
                        for half in range(2):
                            nc.tensor.matmul(
                                kv_ps[:, half * 512:(half + 1) * 512],
                                lhsT=xfull[:, m, kk, :],
                                rhs=wkv_sb[:, kk, half * 512:(half + 1) * 512],
                                start=(kk == 0), stop=(kk == 7))

                    expk = work.tile([128, INNER], BF, tag="expk")
                    nc.scalar.activation(out=expk, in_=kv_ps[:, 0:512], func=EXP)
                    vsb = work.tile([128, INNER], BF, tag="v")
                    nc.vector.tensor_copy(out=vsb, in_=kv_ps[:, 512:1024])

                    # per-tile single-shot ctx/Z matmuls, accumulate in SBUF
                    cz = ctx_psum.tile([128, 260], F32, tag="cz")
                    for h in range(H):
                        t, r = h // 2, h % 2
                        nc.tensor.matmul(
                            cz[64 * r:64 * (r + 1), 64 * t:64 * (t + 1)],
                            lhsT=vsb[:, 64 * h:64 * (h + 1)],
                            rhs=expk[:, 64 * h:64 * (h + 1)],
                            start=True, stop=True)
                    # Z4: col sums of exp_k; dims 128j..128j+127 -> col 256+j
                    for j in range(4):
                        nc.tensor.matmul(
                            cz[:, 256 + j:257 + j],
                            lhsT=expk[:, 128 * j:128 * (j + 1)], rhs=ones_bf,
                            start=True, stop=True)
                    nc.vector.tensor_add(cz_acc, cz_acc, cz)

                # Replicate czb 8x and AllToAll it: same wire bytes as an
                # AllGather, but the runtime schedules A2A as a one-hop
                # exchange instead of multi-stage RDH. The later tree-sum is
                # permutation-invariant, so the block order doesn't matter.
                czr = small.tile([128, NCORES, 260], BF, tag="czr")
                for i in range(NCORES):
                    nc.vector.tensor_copy(out=czr[:, i, :], in_=cz_acc)
                part_b = dram.tile([NCORES * 128, 260], BF, tag=f"part{b}")
                red_b = dram.tile([NCORES * 128, 260], BF, tag=f"red{b}")
                nc.sync.dma_start(
                    out=part_b[:].rearrange("(r p) c -> p r c", p=128), in_=czr)
                nc.gpsimd.collective_compute(
                    "AllToAll", mybir.AluOpType.bypass,
                    replica_groups=[list(range(NCORES))],
                    ins=[part_b.opt()], outs=[red_b.opt()])
                red.append(red_b)

        # phase Q: q^T GEMM + softmax in transposed layout (local only;
        # overlaps the AllGathers). Chunks 0,1 cover batch 0.
        with ExitStack() as pq:
            eq_pool = pq.enter_context(tc.tile_pool(name="eq", bufs=9))
            rq_pool = pq.enter_context(tc.tile_pool(name="rq", bufs=2))
            qt_psum = pq.enter_context(tc.tile_pool(name="qt_ps", bufs=2, space="PSUM"))
            qs_psum = pq.enter_context(tc.tile_pool(name="qs_ps", bufs=2, space="PSUM"))
            bc_psum = pq.enter_context(tc.tile_pool(name="bc_ps", bufs=2, space="PSUM"))

            # One-chunk software pipeline: chunk c's broadcast+normalize is
            # emitted after chunk c+1's GEMMs so the PE never stalls on the
            # (slow, iterative-divide) reciprocal.
            def finalize(c, eqs, rq):
                for t in range(4):
                    bc_ps = bc_psum.tile([128, 512], F32, tag="bc")
                    nc.tensor.matmul(bc_ps, lhsT=bsel[:, t, :], rhs=rq,
                                     start=True, stop=True)
                    nc.vector.tensor_mul(
                        qsmT[:, t, 512 * c:512 * (c + 1)], eqs[t], bc_ps)

            pend = None
            for c in range(NCH):
                eqs = []
                qsum_ps = qs_psum.tile([8, 512], F32, tag="qsum")
                for t in range(4):
                    qt_ps = qt_psum.tile([128, 512], F32, tag="qt")
                    for kk in range(8):
                        nc.tensor.matmul(
                            qt_ps,
                            lhsT=wq_sb[:, kk, 128 * t:128 * (t + 1)],
                            rhs=xfull[:, 4 * c:4 * (c + 1), kk, :],
                            start=(kk == 0), stop=(kk == 7))
                    eqT = eq_pool.tile([128, 512], BF, tag="eq")
                    nc.scalar.activation(out=eqT, in_=qt_ps, func=EXP)
                    eqs.append(eqT)
                for t in range(4):
                    nc.tensor.matmul(
                        qsum_ps, lhsT=osel[:, t, :], rhs=eqs[t],
                        start=(t == 0), stop=(t == 3))
                rq = rq_pool.tile([8, 512], BF, tag="rq")
                with nc.allow_low_precision(reason="1/qsum broadcast in bf16"):
                    nc.vector.reciprocal(rq, qsum_ps)
                if pend is not None:
                    finalize(*pend)
                pend = (c, eqs, rq)
            finalize(*pend)

        with ExitStack() as p45:
            work2 = p45.enter_context(tc.tile_pool(name="work2", bufs=2))
            small2 = p45.enter_context(tc.tile_pool(name="small2", bufs=2))
            ysb_pool = p45.enter_context(tc.tile_pool(name="ysb", bufs=4))
            m_psum = p45.enter_context(tc.tile_pool(name="m_ps", bufs=2, space="PSUM"))
            y_psum = p45.enter_context(tc.tile_pool(name="y_ps", bufs=4, space="PSUM"))

            for b in range(B):
                allcz = work2.tile([128, 8, 260], BF, tag="allcz")
                nc.sync.dma_start(
                    out=allcz,
                    in_=red[b][:].rearrange("(r p) c -> p r c", p=128))
                # tree-sum the 8 shards in f32
                s0 = work2.tile([128, 260], F32, tag="s0")
                s1 = work2.tile([128, 260], F32, tag="s1")
                s2 = work2.tile([128, 260], F32, tag="s2")
                s3 = work2.tile([128, 260], F32, tag="s3")
                nc.vector.tensor_add(s0, allcz[:, 0, :], allcz[:, 1, :])
                nc.vector.tensor_add(s1, allcz[:, 2, :], allcz[:, 3, :])
                nc.vector.tensor_add(s2, allcz[:, 4, :], allcz[:, 5, :])
                nc.vector.tensor_add(s3, allcz[:, 6, :], allcz[:, 7, :])
                nc.vector.tensor_add(s0, s0, s1)
                nc.vector.tensor_add(s2, s2, s3)
                red_sb = work2.tile([128, 260], F32, tag="red")
                nc.vector.tensor_add(red_sb, s0, s2)

                ctxbf = work2.tile([128, 256], BF, tag="ctxbf")
                nc.vector.tensor_copy(out=ctxbf, in_=red_sb[:, 0:256])
                rz = small2.tile([128, 4], F32, tag="rz")
                nc.vector.reciprocal(rz, red_sb[:, 256:260])

                # fold 1/Z into q_sm^T (per-partition scalar per 128-dim region)
                qsmZ = work2.tile([128, 4, NL], BF, tag="qsmz")
                for t in range(4):
                    nc.vector.tensor_scalar_mul(
                        qsmZ[:, t, :], qsmT[:, t, b * NL:(b + 1) * NL], rz[:, t:t + 1])

                # M_h = ctx_h @ Wout_h ; M region t rows = dims 128t..128t+127
                m_sb = work2.tile([128, 4, D], BF, tag="msb")
                for t in range(4):
                    for cb in range(2):
                        mp = m_psum.tile([128, 512], F32, tag="mp")
                        for r in range(2):
                            nc.tensor.matmul(
                                mp[64 * r:64 * (r + 1), :],
                                lhsT=ctxbf[64 * r:64 * (r + 1), 64 * t:64 * (t + 1)],
                                rhs=wo_sb[64 * r:64 * (r + 1), t, cb * 512:(cb + 1) * 512],
                                start=True, stop=True)
                        nc.vector.tensor_copy(
                            out=m_sb[:, t, cb * 512:(cb + 1) * 512], in_=mp)

                # y = sum_t qsmZ_t^T @ M_t
                for mi in range(NT_B):
                    for cb in range(2):
                        yp = y_psum.tile([128, 512], F32, tag="yp")
                        for t in range(4):
                            nc.tensor.matmul(
                                yp, lhsT=qsmZ[:, t, mi * 128:(mi + 1) * 128],
                                rhs=m_sb[:, t, cb * 512:(cb + 1) * 512],
                                start=(t == 0), stop=(t == 3))
                        ysb = ysb_pool.tile([128, 512], BF, tag="ysb")
                        nc.scalar.copy(out=ysb, in_=yp)
                        nc.sync.dma_start(
                            out=y[b * NL + mi * 128: b * NL + (mi + 1) * 128,
                                  cb * 512:(cb + 1) * 512],
                            in_=ysb)


_COMPILED = None


def _build():
    global _COMPILED
    if _COMPILED is None:
        nc = bacc.Bacc("TRN2", target_bir_lowering=False, debug=False,
                       num_devices=NCORES)
        xt = nc.declare_dram_parameter("xt", [128, NT * 1024], BF, isOutput=False)
        wq = nc.declare_dram_parameter("wq", [D, INNER], BF, isOutput=False)
        wkv = nc.declare_dram_parameter("wkv", [D, 2 * INNER], BF, isOutput=False)
        wo = nc.declare_dram_parameter("wo", [INNER, D], BF, isOutput=False)
        osel_d = nc.declare_dram_parameter("osel", [128, 32], BF, isOutput=False)
        bsel_d = nc.declare_dram_parameter("bsel", [8, 512], BF, isOutput=False)
        y = nc.declare_dram_parameter("y", [SEQ, D], BF, isOutput=True)
        with tile.TileContext(nc) as tc:
            _body(tc, xt, wq, wkv, wo, osel_d, bsel_d, y)
        nc.compile()
        _COMPILED = nc
    return _COMPILED


def _sel_arrays():
    osel = np.zeros((128, 4, 8), np.float32)
    bsel = np.zeros((8, 4, 128), np.float32)
    for t in range(4):
        osel[0:64, t, 2 * t] = 1.0
        osel[64:128, t, 2 * t + 1] = 1.0
        bsel[2 * t, t, 0:64] = 1.0
        bsel[2 * t + 1, t, 64:128] = 1.0
    return (osel.reshape(128, 32).astype(bf16),
            bsel.reshape(8, 512).astype(bf16))


def _make_in_maps(x, W_qkv, W_out):
    osel_np, bsel_np = _sel_arrays()
    wq_bf = np.ascontiguousarray(W_qkv[:, 0:INNER]).astype(bf16)
    wkv_bf = np.ascontiguousarray(W_qkv[:, INNER:]).astype(bf16)
    wo_bf = np.ascontiguousarray(W_out).astype(bf16)
    in_maps = []
    for c in range(NCORES):
        rows = slice(c * NL, (c + 1) * NL)
        xs = np.concatenate([x[0, rows], x[1, rows]], axis=0)  # [2048, 1024]
        # [seq, d] -> [d-part 128, tile 16, ktile 8, s 128] flattened
        xtl = xs.reshape(NT, 128, 8, 128).transpose(3, 0, 2, 1)
        xtl = np.ascontiguousarray(xtl).astype(bf16).reshape(128, NT * 1024)
        in_maps.append({"xt": xtl, "wq": wq_bf, "wkv": wkv_bf, "wo": wo_bf,
                        "osel": osel_np, "bsel": bsel_np})
    return in_maps


def _run(x, W_qkv, W_out, b_out, trace=False, **spmd_kwargs):
    nc = _build()
    in_maps = _make_in_maps(x, W_qkv, W_out)
    res = run_bass_kernel_spmd(nc, in_maps, list(range(NCORES)),
                               trace=trace, **spmd_kwargs)
    out = np.empty((B, N, D), np.float32)
    for c in range(NCORES):
        yc = np.asarray(res.results[c]["y"]).astype(np.float32)
        rows = slice(c * NL, (c + 1) * NL)
        out[0, rows] = yc[:NL]
        out[1, rows] = yc[NL:]
    out += np.asarray(b_out, np.float32)[None, None, :]
    return out, res


def kernel(x, W_qkv, W_out, b_out):
    x = np.asarray(x, np.float32)
    out, _ = _run(x, np.asarray(W_qkv, np.float32),
                  np.asarray(W_out, np.float32),
                  np.asarray(b_out, np.float32))
    return out
